# revision 1
# baseline (speedup 1.0000x reference)
"""Trainium2 Bass kernel for CosmicNetGNN (NNConv message passing).

Strategy: shard nodes into 8 contiguous dst-bands (2048 nodes/core); host
sorts edges by dst so each core owns all in-edges of its band.  Per layer,
each core builds per-edge outer products z[e,(i,k)] = h[src[e],i]*e2[e,k] on
the vector engine, scatters them into per-node-tile S matrices with
indicator matmuls on the PE (contracting edges), applies the fixed
reorganized ew3 matrix M[(i,k),o] node-side (4x fewer FLOPs than per-edge),
then LayerNorm + residual locally and AllGathers the h bands.
The edge MLP depends only on edge_attr, so it is precomputed for all layers.
"""
import sys
sys.path.insert(0, '/opt/trn_rl_repo')
import numpy as np
from concourse import bass, mybir, tile, bacc
from concourse import bass_utils
from concourse.masks import make_identity

N, E, B = 16384, 65536, 32
D_IN, ED, H, L = 4, 5, 64, 3
NEG = 0.1
EPS = 1e-5
NC = 8
BAND = N // NC       # 2048
NT = BAND // 128     # 16 node tiles per core
F32 = mybir.dt.float32
I32 = mybir.dt.int32


def _leaky_np(v):
    return np.where(v > 0, v, NEG * v)


def _host_prep(inputs):
    x = np.asarray(inputs['x'], np.float32)
    ei = np.asarray(inputs['edge_index']).astype(np.int64)
    ea = np.asarray(inputs['edge_attr'], np.float32)
    batch = np.asarray(inputs['batch']).astype(np.int64)
    src, dst = ei[0], ei[1]

    deg = np.bincount(dst, minlength=N).astype(np.float32)
    deg_inv = (1.0 / np.maximum(deg, 1.0)).astype(np.float32)

    order = np.argsort(dst, kind='stable')
    src_s, dst_s, ea_s = src[order], dst[order], ea[order]
    gt = dst_s // 128
    counts = np.bincount(gt, minlength=N // 128)
    T_et = int(np.ceil(counts.max() / 128))
    EP = NT * T_et * 128

    src_pad = np.zeros((NC, EP), np.int32)
    dstloc_pad = np.full((NC, EP), -1.0, np.float32)
    ea_pad = np.zeros((NC, EP, ED), np.float32)
    starts = np.concatenate([[0], np.cumsum(counts)])
    for c in range(NC):
        for t in range(NT):
            g = c * NT + t
            s, e = starts[g], starts[g + 1]
            cnt = e - s
            o = t * T_et * 128
            src_pad[c, o:o + cnt] = src_s[s:e]
            dstloc_pad[c, o:o + cnt] = (dst_s[s:e] - g * 128).astype(np.float32)
            ea_pad[c, o:o + cnt] = ea_s[s:e]

    eaT1 = np.concatenate([ea_pad.transpose(0, 2, 1),
                           np.ones((NC, 1, EP), np.float32)], axis=1)  # [NC,6,EP]

    cnt_b = np.bincount(batch, minlength=B).astype(np.float32)
    onehot_sc = np.zeros((N, B), np.float32)
    onehot_sc[np.arange(N), batch] = 1.0 / np.maximum(cnt_b, 1.0)[batch]

    ew3 = np.asarray(inputs['ew3'], np.float32)
    eb3 = np.asarray(inputs['eb3'], np.float32)
    NCH = H * H // 128 + 1            # 33 chunks of the [4160,64] M_aug
    M_pack = np.zeros((L, 128, NCH * H), np.float32)
    for l in range(L):
        w = ew3[l].reshape(H, H, H)                   # [i, o, k]
        M_aug = np.zeros((NCH * 128, H), np.float32)
        M_aug[:H * H] = w.transpose(0, 2, 1).reshape(H * H, H)   # [(i,k), o]
        M_aug[H * H:H * H + H] = eb3[l].reshape(H, H)            # [i, o]
        for q in range(NCH):
            M_pack[l, :, q * H:(q + 1) * H] = M_aug[q * 128:(q + 1) * 128]

    def aug(wT, b):  # [K,H']+[H'] -> [K+1,H']
        return np.concatenate([np.asarray(wT, np.float32),
                               np.asarray(b, np.float32)[None, :]], 0)

    host = dict(
        T_et=T_et, EP=EP,
        deg_inv_t=np.stack([deg_inv[c * BAND:(c + 1) * BAND]
                            .reshape(NT, 128).T for c in range(NC)]),   # [NC,128,NT]
        src_idx=np.stack([src_pad[c].reshape(-1, 128).T
                          for c in range(NC)]),                         # [NC,128,ET]
        dstloc=np.stack([dstloc_pad[c].reshape(-1, 128).T
                         for c in range(NC)]),                          # [NC,128,ET]
        own_idx=np.stack([np.arange(c * BAND, (c + 1) * BAND, dtype=np.int32)
                          .reshape(NT, 128).T for c in range(NC)]),     # [NC,128,NT]
        eaT1=eaT1,
        onehot_sc=onehot_sc,
        M_pack=M_pack,
        xT1=np.concatenate([x.T, np.ones((1, N), np.float32)], 0),      # [5,N]
        win_rhs=aug(np.asarray(inputs['W_in']).T, inputs['b_in']),      # [5,64]
        ew1_rhs=np.stack([aug(np.asarray(inputs['ew1'][l]).T, inputs['eb1'][l])
                          for l in range(L)]),                          # [L,6,64]
        ew2_rhs=np.stack([aug(np.asarray(inputs['ew2'][l]).T, inputs['eb2'][l])
                          for l in range(L)]),                          # [L,65,64]
        root_rhs=np.stack([aug(np.asarray(inputs['root_w'][l]).T, inputs['root_b'][l])
                           for l in range(L)]),                         # [L,65,64]
        ln_g=np.broadcast_to(np.asarray(inputs['ln_g'], np.float32)[:, None, :],
                             (L, 128, H)).copy(),                       # [L,128,64]
        ln_b=np.broadcast_to(np.asarray(inputs['ln_b'], np.float32)[:, None, :],
                             (L, 128, H)).copy(),
        pw1_rhs=aug(np.asarray(inputs['pw1']).T, inputs['pb1']),        # [65,64]
        pw2_rhs=aug(np.asarray(inputs['pw2']).T, inputs['pb2']),        # [65,32]
        pw3_rhs=aug(np.asarray(inputs['pw3']).T, inputs['pb3']),        # [33,1]
        iota=np.broadcast_to(np.arange(128, dtype=np.float32)[None, :],
                             (128, 128)).copy(),
    )
    return host


def _leaky(nc, pool, out_ap, in_ap, shape):
    """out = max(in, 0.1*in) — safe leaky relu via 2 DVE ops."""
    tmp = pool.tile(list(shape), F32, tag='lk_tmp')
    nc.vector.tensor_scalar_mul(tmp[:], in_ap, NEG)
    nc.vector.tensor_tensor(out=out_ap, in0=tmp[:], in1=in_ap,
                            op=mybir.AluOpType.max)


def _build(T_et, EP):
    ET = NT * T_et  # edge tiles per core
    NCH = H * H // 128 + 1
    nc = bacc.Bacc('TRN2', target_bir_lowering=False, debug=False,
                   num_devices=NC)

    def din(name, shape, dt=F32):
        return nc.dram_tensor(name, list(shape), dt, kind='ExternalInput')

    t_xT1 = din('xT1', [D_IN + 1, N])
    t_win = din('win_rhs', [D_IN + 1, H])
    t_eaT1 = din('eaT1', [ED + 1, EP])
    t_ew1 = din('ew1_rhs', [L, ED + 1, H])
    t_ew2 = din('ew2_rhs', [L, H + 1, H])
    t_root = din('root_rhs', [L, H + 1, H])
    t_M = din('M_pack', [L, 128, NCH * H])
    t_lng = din('ln_g', [L, 128, H])
    t_lnb = din('ln_b', [L, 128, H])
    t_deg = din('deg_inv_t', [128, NT])
    t_srci = din('src_idx', [128, ET], I32)
    t_dstl = din('dstloc', [128, ET])
    t_owni = din('own_idx', [128, NT], I32)
    t_oh = din('onehot_sc', [N, B])
    t_iota = din('iota', [128, 128])
    t_pw1 = din('pw1_rhs', [H + 1, H])
    t_pw2 = din('pw2_rhs', [H + 1, B])
    t_pw3 = din('pw3_rhs', [B + 1, 1])
    t_out = nc.dram_tensor('pred', [1, B], F32, kind='ExternalOutput')

    with tile.TileContext(nc) as tc:
        with (tc.tile_pool(name='const', bufs=1) as cp,
              tc.tile_pool(name='dram', bufs=1, space='DRAM') as dram):
            ident = cp.tile([128, 128], F32)
            make_identity(nc, ident[:])
            iota_sb = cp.tile([128, 128], F32)
            nc.sync.dma_start(out=iota_sb[:], in_=t_iota[:, :])
            deg_sb = cp.tile([128, NT], F32)
            nc.sync.dma_start(out=deg_sb[:], in_=t_deg[:, :])
            srci_sb = cp.tile([128, ET], I32)
            nc.sync.dma_start(out=srci_sb[:], in_=t_srci[:, :])
            dstl_sb = cp.tile([128, ET], F32)
            nc.sync.dma_start(out=dstl_sb[:], in_=t_dstl[:, :])
            owni_sb = cp.tile([128, NT], I32)
            nc.sync.dma_start(out=owni_sb[:], in_=t_owni[:, :])
            M_sb = [cp.tile([128, NCH * H], F32, name=f'Msb{l}', tag=f'M{l}') for l in range(L)]
            for l in range(L):
                nc.sync.dma_start(out=M_sb[l][:], in_=t_M[l, :, :])
            root_sb = [cp.tile([H + 1, H], F32, name=f'rtsb{l}', tag=f'rt{l}') for l in range(L)]
            lng_sb = [cp.tile([128, H], F32, name=f'lgsb{l}', tag=f'lg{l}') for l in range(L)]
            lnb_sb = [cp.tile([128, H], F32, name=f'lbsb{l}', tag=f'lb{l}') for l in range(L)]
            for l in range(L):
                nc.sync.dma_start(out=root_sb[l][:], in_=t_root[l, :, :])
                nc.sync.dma_start(out=lng_sb[l][:], in_=t_lng[l, :, :])
                nc.sync.dma_start(out=lnb_sb[l][:], in_=t_lnb[l, :, :])

            h_tab = [dram.tile([N, H], F32, name=f'htab{i}', tag=f'h{i}') for i in range(L + 1)]
            band_d = [dram.tile([BAND, H], F32, name=f'bandd{l}', tag=f'bd{l}') for l in range(L)]
            e2_d = [dram.tile([EP, H], F32, name=f'e2d{l}', tag=f'e2{l}') for l in range(L)]

            # ---- stage 0: input projection h0 = leaky(x @ W_in.T + b) ----
            with (tc.tile_pool(name='s0', bufs=3) as s0,
                  tc.tile_pool(name='s0c', bufs=1) as s0c,
                  tc.tile_pool(name='s0p', bufs=3, space='PSUM') as s0p):
                xT_sb = s0c.tile([D_IN + 1, N], F32)
                nc.sync.dma_start(out=xT_sb[:], in_=t_xT1[:, :])
                win_sb = s0c.tile([D_IN + 1, H], F32)
                nc.sync.dma_start(out=win_sb[:], in_=t_win[:, :])
                for g in range(N // 128):
                    ps = s0p.tile([128, H], F32, tag='p')
                    nc.tensor.matmul(out=ps[:], lhsT=xT_sb[:, g * 128:(g + 1) * 128],
                                     rhs=win_sb[:], start=True, stop=True)
                    h0t = s0.tile([128, H], F32, tag='h0')
                    _leaky(nc, s0, h0t[:], ps[:], (128, H))
                    nc.sync.dma_start(out=h_tab[0][g * 128:(g + 1) * 128, :],
                                      in_=h0t[:])

            # ---- stage 0b: edge MLP e2 for all layers ----
            with (tc.tile_pool(name='em', bufs=3) as em,
                  tc.tile_pool(name='emc', bufs=1) as emc,
                  tc.tile_pool(name='emp', bufs=3, space='PSUM') as emp):
                ea_sb = emc.tile([ED + 1, EP], F32)
                nc.sync.dma_start(out=ea_sb[:], in_=t_eaT1[:, :])
                e1_sb = emc.tile([H + 1, EP], F32)
                nc.vector.memset(e1_sb[H:H + 1, :], 1.0)
                for l in range(L):
                    w1 = em.tile([ED + 1, H], F32, tag='w1')
                    nc.sync.dma_start(out=w1[:], in_=t_ew1[l, :, :])
                    w2 = em.tile([H + 1, H], F32, tag='w2')
                    nc.sync.dma_start(out=w2[:], in_=t_ew2[l, :, :])
                    for q in range(EP // 512):
                        ps1 = emp.tile([H, 512], F32, tag='p1')
                        nc.tensor.matmul(out=ps1[:],
                                         lhsT=w1[:],
                                         rhs=ea_sb[:, q * 512:(q + 1) * 512],
                                         start=True, stop=True)
                        _leaky(nc, em, e1_sb[0:H, q * 512:(q + 1) * 512], ps1[:],
                               (H, 512))
                    for et in range(ET):
                        ps2 = emp.tile([128, H], F32, tag='p2')
                        nc.tensor.matmul(out=ps2[:],
                                         lhsT=e1_sb[:, et * 128:(et + 1) * 128],
                                         rhs=w2[:], start=True, stop=True)
                        e2t = em.tile([128, H], F32, tag='e2t')
                        _leaky(nc, em, e2t[:], ps2[:], (128, H))
                        nc.sync.dma_start(out=e2_d[l][et * 128:(et + 1) * 128, :],
                                          in_=e2t[:])

            # ---- layers ----
            with (tc.tile_pool(name='zz', bufs=T_et + 2) as zp,
                  tc.tile_pool(name='ly', bufs=2 * T_et + 2) as ly,
                  tc.tile_pool(name='lyn', bufs=3) as lyn,
                  tc.tile_pool(name='ssb', bufs=2) as ssb,
                  tc.tile_pool(name='pS', bufs=3, space='PSUM') as pS,
                  tc.tile_pool(name='pHs', bufs=1, space='PSUM') as pHs,
                  tc.tile_pool(name='ptp', bufs=2, space='PSUM') as ptp,
                  tc.tile_pool(name='pag', bufs=1, space='PSUM') as pag,
                  tc.tile_pool(name='pag2', bufs=1, space='PSUM') as pag2):
                for l in range(L):
                    hin = h_tab[l]
                    for nt in range(NT):
                        z_t, ind_t, hs_t = [], [], []
                        for j in range(T_et):
                            et = nt * T_et + j
                            hs = ly.tile([128, H], F32, tag='hs')
                            nc.gpsimd.indirect_dma_start(
                                out=hs[:], out_offset=None,
                                in_=hin[:, :],
                                in_offset=bass.IndirectOffsetOnAxis(
                                    ap=srci_sb[:, et:et + 1], axis=0))
                            e2t = ly.tile([128, H], F32, tag='e2')
                            nc.sync.dma_start(
                                out=e2t[:],
                                in_=e2_d[l][et * 128:(et + 1) * 128, :])
                            z = zp.tile([128, H * H], F32, tag='z')
                            zv = z[:].rearrange('p (i k) -> p i k', i=H)
                            SP = 40  # DVE gets 40/64 of z, idle GPSIMD the rest
                            nc.vector.tensor_tensor(
                                out=zv[:, 0:SP, :],
                                in0=e2t[:][:, None, :].to_broadcast([128, SP, H]),
                                in1=hs[:][:, 0:SP].to_broadcast([128, SP, H]),
                                op=mybir.AluOpType.mult)
                            nc.gpsimd.tensor_tensor(
                                out=zv[:, SP:H, :],
                                in0=e2t[:][:, None, :].to_broadcast([128, H - SP, H]),
                                in1=hs[:][:, SP:H].to_broadcast([128, H - SP, H]),
                                op=mybir.AluOpType.mult)
                            ind = ly.tile([128, 128], F32, tag='ind')
                            nc.vector.tensor_tensor(
                                out=ind[:],
                                in0=dstl_sb[:, et:et + 1].to_broadcast([128, 128]),
                                in1=iota_sb[:],
                                op=mybir.AluOpType.is_equal)
                            z_t.append(z); ind_t.append(ind); hs_t.append(hs)

                        S_sb = ssb.tile([128, H * H + H], F32, tag='S')
                        # Hs chunk (scatter of h_src)
                        hps = pHs.tile([128, H], F32, tag='hs')
                        for j in range(T_et):
                            nc.tensor.matmul(out=hps[:], lhsT=ind_t[j][:],
                                             rhs=hs_t[j][:],
                                             start=(j == 0), stop=(j == T_et - 1))
                        nc.scalar.copy(out=S_sb[:, H * H:], in_=hps[:])
                        # 8 chunks of 512 in 4 phases of 2
                        for ph in range(4):
                            for c2 in range(2):
                                q = 2 * ph + c2
                                sp = pS.tile([128, 512], F32, tag='sp')
                                for j in range(T_et):
                                    nc.tensor.matmul(
                                        out=sp[:], lhsT=ind_t[j][:],
                                        rhs=z_t[j][:, q * 512:(q + 1) * 512],
                                        start=(j == 0), stop=(j == T_et - 1))
                                nc.scalar.copy(
                                    out=S_sb[:, q * 512:(q + 1) * 512], in_=sp[:])
                        # agg = S_aug @ M_aug  via PE transposes of S chunks
                        agg = pag.tile([128, H], F32, tag='agg')
                        for q in range(NCH):
                            kk = 128 if q < NCH - 1 else H
                            tp = ptp.tile([128, 128], F32, tag='tp')
                            nc.tensor.transpose(
                                out=tp[0:kk, :],
                                in_=S_sb[:, q * 128:q * 128 + kk],
                                identity=ident[:])
                            st = lyn.tile([128, 128], F32, tag='st')
                            nc.scalar.copy(out=st[0:kk, :], in_=tp[0:kk, :])
                            nc.tensor.matmul(out=agg[:], lhsT=st[0:kk, :],
                                             rhs=M_sb[l][0:kk, q * H:(q + 1) * H],
                                             start=(q == 0), stop=(q == NCH - 1))
                        # own h: gather + root matmul
                        ho = lyn.tile([128, H], F32, tag='ho')
                        nc.gpsimd.indirect_dma_start(
                            out=ho[:], out_offset=None, in_=hin[:, :],
                            in_offset=bass.IndirectOffsetOnAxis(
                                ap=owni_sb[:, nt:nt + 1], axis=0))
                        htp = ptp.tile([128, 128], F32, tag='tp')
                        nc.tensor.transpose(out=htp[0:H, :], in_=ho[:],
                                            identity=ident[:])
                        hoT = lyn.tile([H + 1, 128], F32, tag='hoT')
                        nc.scalar.copy(out=hoT[0:H, :], in_=htp[0:H, :])
                        nc.vector.memset(hoT[H:H + 1, :], 1.0)
                        ag2 = pag2.tile([128, H], F32, tag='ag2')
                        nc.tensor.matmul(out=ag2[:], lhsT=hoT[:], rhs=root_sb[l][:],
                                         start=True, stop=True)
                        # node update: out = agg*deg_inv + root; LN; leaky; +h
                        xs = lyn.tile([128, H], F32, tag='xs')
                        nc.vector.tensor_scalar(
                            out=xs[:], in0=agg[:], scalar1=deg_sb[:, nt:nt + 1],
                            scalar2=None, op0=mybir.AluOpType.mult)
                        nc.vector.tensor_tensor(out=xs[:], in0=xs[:], in1=ag2[:],
                                                op=mybir.AluOpType.add)
                        mu = lyn.tile([128, 1], F32, tag='mu')
                        nc.vector.tensor_reduce(out=mu[:], in_=xs[:],
                                                axis=mybir.AxisListType.X,
                                                op=mybir.AluOpType.add)
                        nc.vector.tensor_scalar_mul(mu[:], mu[:], 1.0 / H)
                        xc = lyn.tile([128, H], F32, tag='xc')
                        nc.vector.tensor_scalar(
                            out=xc[:], in0=xs[:], scalar1=mu[:], scalar2=None,
                            op0=mybir.AluOpType.subtract)
                        sq = lyn.tile([128, H], F32, tag='sq')
                        nc.vector.tensor_tensor(out=sq[:], in0=xc[:], in1=xc[:],
                                                op=mybir.AluOpType.mult)
                        vs = lyn.tile([128, 1], F32, tag='vs')
                        nc.vector.tensor_reduce(out=vs[:], in_=sq[:],
                                                axis=mybir.AxisListType.X,
                                                op=mybir.AluOpType.add)
                        nc.vector.tensor_scalar(
                            out=vs[:], in0=vs[:], scalar1=1.0 / H, scalar2=EPS,
                            op0=mybir.AluOpType.mult, op1=mybir.AluOpType.add)
                        sd = lyn.tile([128, 1], F32, tag='sd')
                        nc.scalar.activation(out=sd[:], in_=vs[:],
                                             func=mybir.ActivationFunctionType.Sqrt)
                        rs = lyn.tile([128, 1], F32, tag='rs')
                        nc.vector.reciprocal(out=rs[:], in_=sd[:])
                        yv = lyn.tile([128, H], F32, tag='yv')
                        nc.vector.tensor_scalar(
                            out=yv[:], in0=xc[:], scalar1=rs[:], scalar2=None,
                            op0=mybir.AluOpType.mult)
                        nc.vector.tensor_tensor(out=yv[:], in0=yv[:],
                                                in1=lng_sb[l][:],
                                                op=mybir.AluOpType.mult)
                        nc.vector.tensor_tensor(out=yv[:], in0=yv[:],
                                                in1=lnb_sb[l][:],
                                                op=mybir.AluOpType.add)
                        hn = lyn.tile([128, H], F32, tag='hn')
                        _leaky(nc, lyn, hn[:], yv[:], (128, H))
                        nc.vector.tensor_tensor(out=hn[:], in0=hn[:], in1=ho[:],
                                                op=mybir.AluOpType.add)
                        nc.sync.dma_start(
                            out=band_d[l][nt * 128:(nt + 1) * 128, :], in_=hn[:])
                    nc.gpsimd.collective_compute(
                        'AllGather', mybir.AluOpType.bypass,
                        replica_groups=[list(range(NC))],
                        ins=[band_d[l][:].opt()],
                        outs=[h_tab[l + 1][:].opt()])

            # ---- pool + head (replicated on all cores) ----
            with (tc.tile_pool(name='hd', bufs=3) as hd,
                  tc.tile_pool(name='hdc', bufs=1) as hdc,
                  tc.tile_pool(name='hdp', bufs=2, space='PSUM') as hdp):
                pool_ps = hdp.tile([H, B], F32, tag='pool')
                for g in range(N // 128):
                    ht = hd.tile([128, H], F32, tag='ht')
                    nc.sync.dma_start(out=ht[:],
                                      in_=h_tab[L][g * 128:(g + 1) * 128, :])
                    oh = hd.tile([128, B], F32, tag='oh')
                    nc.sync.dma_start(out=oh[:], in_=t_oh[g * 128:(g + 1) * 128, :])
                    nc.tensor.matmul(out=pool_ps[:], lhsT=ht[:], rhs=oh[:],
                                     start=(g == 0), stop=(g == N // 128 - 1))
                pT = hdc.tile([H + 1, B], F32)
                nc.scalar.copy(out=pT[0:H, :], in_=pool_ps[:])
                nc.vector.memset(pT[H:H + 1, :], 1.0)
                w1 = hdc.tile([H + 1, H], F32)
                nc.sync.dma_start(out=w1[:], in_=t_pw1[:, :])
                w2 = hdc.tile([H + 1, B], F32)
                nc.sync.dma_start(out=w2[:], in_=t_pw2[:, :])
                w3 = hdc.tile([B + 1, 1], F32)
                nc.sync.dma_start(out=w3[:], in_=t_pw3[:, :])
                p1ps = hdp.tile([H, B], F32, tag='p1')
                nc.tensor.matmul(out=p1ps[:], lhsT=w1[:], rhs=pT[:],
                                 start=True, stop=True)
                p1 = hdc.tile([H + 1, B], F32)
                _leaky(nc, hd, p1[0:H, :], p1ps[:], (H, B))
                nc.vector.memset(p1[H:H + 1, :], 1.0)
                p2ps = hdp.tile([B, B], F32, tag='p2')
                nc.tensor.matmul(out=p2ps[:], lhsT=w2[:], rhs=p1[:],
                                 start=True, stop=True)
                p2 = hdc.tile([B + 1, B], F32)
                _leaky(nc, hd, p2[0:B, :], p2ps[:], (B, B))
                nc.vector.memset(p2[B:B + 1, :], 1.0)
                p3ps = hdp.tile([1, B], F32, tag='p3')
                nc.tensor.matmul(out=p3ps[:], lhsT=w3[:], rhs=p2[:],
                                 start=True, stop=True)
                pr = hdc.tile([1, B], F32)
                nc.scalar.copy(out=pr[:], in_=p3ps[:])
                nc.sync.dma_start(out=t_out[:, :], in_=pr[:])

    nc.compile()
    return nc


_CACHE = {}


def kernel(**inputs) -> np.ndarray:
    host = _host_prep(inputs)
    T_et, EP = host['T_et'], host['EP']
    key = (T_et, EP)
    if key not in _CACHE:
        _CACHE[key] = _build(T_et, EP)
    nc = _CACHE[key]

    shared = dict(
        xT1=host['xT1'], win_rhs=host['win_rhs'], ew1_rhs=host['ew1_rhs'],
        ew2_rhs=host['ew2_rhs'], root_rhs=host['root_rhs'],
        M_pack=host['M_pack'], ln_g=host['ln_g'], ln_b=host['ln_b'],
        onehot_sc=host['onehot_sc'], iota=host['iota'],
        pw1_rhs=host['pw1_rhs'], pw2_rhs=host['pw2_rhs'],
        pw3_rhs=host['pw3_rhs'])
    in_maps = []
    for c in range(NC):
        m = dict(shared)
        m['eaT1'] = host['eaT1'][c]
        m['src_idx'] = host['src_idx'][c]
        m['dstloc'] = host['dstloc'][c]
        m['own_idx'] = host['own_idx'][c]
        m['deg_inv_t'] = host['deg_inv_t'][c]
        in_maps.append({k: np.ascontiguousarray(v) for k, v in m.items()})

    res = bass_utils.run_bass_kernel_spmd(nc, in_maps, core_ids=list(range(NC)))
    return np.asarray(res.results[0]['pred'][0], np.float32)



# revision 8
# speedup vs baseline: 2.4679x; 2.4679x over previous
"""Trainium2 Bass kernel for CosmicNetGNN (NNConv message passing).

Strategy: shard nodes into 8 contiguous dst-bands (2048 nodes/core); host
sorts edges by dst so each core owns all in-edges of its band.  Per layer,
each core builds per-edge outer products z[e,(i,k)] = h[src[e],i]*e2[e,k]
in bf16 on DVE+GPSIMD, scatters them transposed into per-node-tile ST
chunks with bf16 indicator matmuls on the PE (contracting edges), applies
the reorganized ew3 matrix M[(ik),o] node-side, then LayerNorm + residual
(fp32, SBUF-resident own band) and AllGathers a bf16 h table per layer.
The edge MLP depends only on edge_attr, so it is precomputed once for all
layers into SBUF (bf16); dst indicators are also precomputed once.  The
final graph pooling is done on the local band only + a small AllReduce,
so the last layer needs no AllGather.
"""
import sys
sys.path.insert(0, '/opt/trn_rl_repo')
import numpy as np
from concourse import bass, mybir, tile, bacc
from concourse import bass_utils
from concourse.masks import make_identity

N, E, B = 16384, 65536, 32
D_IN, ED, H, L = 4, 5, 64, 3
NEG = 0.1
EPS = 1e-5
NC = 8
BAND = N // NC       # 2048
NT = BAND // 128     # 16 node tiles per core
NCH = H * H // 128 + 1   # 33 chunks of the [4160,64] M_aug
F32 = mybir.dt.float32
BF16 = mybir.dt.bfloat16
I32 = mybir.dt.int32
NP_BF16 = mybir.dt.np(mybir.dt.bfloat16)
SP_ROWS = 42         # z rows on DVE; remaining H-SP_ROWS go to GPSIMD


def _host_prep(inputs):
    x = np.asarray(inputs['x'], np.float32)
    ei = np.asarray(inputs['edge_index']).astype(np.int64)
    ea = np.asarray(inputs['edge_attr'], np.float32)
    batch = np.asarray(inputs['batch']).astype(np.int64)
    src, dst = ei[0], ei[1]

    deg = np.bincount(dst, minlength=N).astype(np.float32)
    deg_inv = (1.0 / np.maximum(deg, 1.0)).astype(np.float32)

    order = np.argsort(dst, kind='stable')
    src_s, dst_s, ea_s = src[order], dst[order], ea[order]
    gt = dst_s // 128
    counts = np.bincount(gt, minlength=N // 128)
    T_et = int(np.ceil(counts.max() / 128))
    EP = NT * T_et * 128

    src_pad = np.zeros((NC, EP), np.int32)
    dstloc_pad = np.full((NC, EP), -1.0, np.float32)
    ea_pad = np.zeros((NC, EP, ED), np.float32)
    starts = np.concatenate([[0], np.cumsum(counts)])
    for c in range(NC):
        for t in range(NT):
            g = c * NT + t
            s, e = starts[g], starts[g + 1]
            cnt = e - s
            o = t * T_et * 128
            src_pad[c, o:o + cnt] = src_s[s:e]
            dstloc_pad[c, o:o + cnt] = (dst_s[s:e] - g * 128).astype(np.float32)
            ea_pad[c, o:o + cnt] = ea_s[s:e]
    # h tables live in half-band-major layout (all cores' first half-bands,
    # then all second half-bands) so each half-band AllGather output is one
    # contiguous block.  Remap gather indices accordingly.
    src_pad = ((src_pad // BAND) * (BAND // 2) + (src_pad % (BAND // 2))
               + (N // 2) * ((src_pad % BAND) // (BAND // 2))).astype(np.int32)

    eaT1 = np.concatenate([ea_pad.transpose(0, 2, 1),
                           np.ones((NC, 1, EP), np.float32)], axis=1)  # [NC,6,EP]

    cnt_b = np.bincount(batch, minlength=B).astype(np.float32)
    onehot_sc = np.zeros((N, B), np.float32)
    onehot_sc[np.arange(N), batch] = 1.0 / np.maximum(cnt_b, 1.0)[batch]
    # per-core own-band pooling matrix: [NC, 128, NT*B]
    oh_own = (onehot_sc.reshape(NC, NT, 128, B).transpose(0, 2, 1, 3)
              .reshape(NC, 128, NT * B).copy())

    ew3 = np.asarray(inputs['ew3'], np.float32)
    eb3 = np.asarray(inputs['eb3'], np.float32)
    M_pack = np.zeros((L, 128, NCH * H), np.float32)
    for l in range(L):
        w = ew3[l].reshape(H, H, H)                   # [i, o, k]
        M_aug = np.zeros((NCH * 128, H), np.float32)
        M_aug[:H * H] = w.transpose(0, 2, 1).reshape(H * H, H)   # [(i,k), o]
        M_aug[H * H:H * H + H] = eb3[l].reshape(H, H)            # [i, o]
        for q in range(NCH):
            M_pack[l, :, q * H:(q + 1) * H] = M_aug[q * 128:(q + 1) * 128]

    xT1 = np.concatenate([x.T, np.ones((1, N), np.float32)], 0)      # [5,N]
    xT_own = np.stack([xT1[:, c * BAND:(c + 1) * BAND] for c in range(NC)])

    def aug(wT, b):  # [K,H']+[H'] -> [K+1,H']
        return np.concatenate([np.asarray(wT, np.float32),
                               np.asarray(b, np.float32)[None, :]], 0)

    host = dict(
        T_et=T_et, EP=EP,
        deg_inv_t=np.stack([deg_inv[c * BAND:(c + 1) * BAND]
                            .reshape(NT, 128).T for c in range(NC)]),   # [NC,128,NT]
        src_idx=np.stack([src_pad[c].reshape(-1, 128).T
                          for c in range(NC)]),                         # [NC,128,ET]
        dstloc=np.stack([dstloc_pad[c].reshape(-1, 128).T
                         for c in range(NC)]),                          # [NC,128,ET]
        eaT1_bf=eaT1.astype(NP_BF16),
        oh_own=oh_own,
        M_pack_bf=M_pack.astype(NP_BF16),
        xT1=xT1,
        xT_own=xT_own,                                                  # [NC,5,BAND]
        win_rhs=aug(np.asarray(inputs['W_in']).T, inputs['b_in']),      # [5,64]
        ew1_bf=np.stack([aug(np.asarray(inputs['ew1'][l]).T, inputs['eb1'][l])
                         for l in range(L)]).astype(NP_BF16),           # [L,6,64]
        ew2_bf=np.stack([aug(np.asarray(inputs['ew2'][l]).T, inputs['eb2'][l])
                         for l in range(L)]).astype(NP_BF16),           # [L,65,64]
        root_rhs=np.stack([aug(np.asarray(inputs['root_w'][l]).T, inputs['root_b'][l])
                           for l in range(L)]),                         # [L,65,64]
        ln_g=np.broadcast_to(np.asarray(inputs['ln_g'], np.float32)[:, None, :],
                             (L, 128, H)).copy(),                       # [L,128,64]
        ln_b=np.broadcast_to(np.asarray(inputs['ln_b'], np.float32)[:, None, :],
                             (L, 128, H)).copy(),
        pw1_rhs=aug(np.asarray(inputs['pw1']).T, inputs['pb1']),        # [65,64]
        pw2_rhs=aug(np.asarray(inputs['pw2']).T, inputs['pb2']),        # [65,32]
        pw3_rhs=aug(np.asarray(inputs['pw3']).T, inputs['pb3']),        # [33,1]
        iota=np.broadcast_to(np.arange(128, dtype=np.float32)[None, :],
                             (128, 128)).copy(),
    )
    return host


SHARED_KEYS = ('xT1', 'win_rhs', 'ew1_bf', 'ew2_bf', 'root_rhs', 'M_pack_bf',
               'ln_g', 'ln_b', 'iota', 'pw1_rhs', 'pw2_rhs', 'pw3_rhs')
PER_CORE_KEYS = ('eaT1_bf', 'src_idx', 'dstloc', 'deg_inv_t', 'oh_own',
                 'xT_own')


def make_in_maps(host):
    in_maps = []
    for c in range(NC):
        m = {k: host[k] for k in SHARED_KEYS}
        for k2 in PER_CORE_KEYS:
            m[k2] = host[k2][c]
        in_maps.append({k2: np.ascontiguousarray(v) for k2, v in m.items()})
    return in_maps


def _leaky(nc, pool, out_ap, in_ap, shape):
    """out = max(in, 0.1*in) — safe leaky relu via 2 DVE ops."""
    tmp = pool.tile(list(shape), F32, tag='lk_tmp')
    nc.vector.tensor_scalar_mul(tmp[:], in_ap, NEG)
    nc.vector.tensor_tensor(out=out_ap, in0=tmp[:], in1=in_ap,
                            op=mybir.AluOpType.max)


def _build(T_et, EP, shared_ag=False):
    ET = NT * T_et  # edge tiles per core
    nc = bacc.Bacc('TRN2', target_bir_lowering=False, debug=False,
                   num_devices=NC)

    def din(name, shape, dt=F32):
        return nc.dram_tensor(name, list(shape), dt, kind='ExternalInput')

    t_xT1 = din('xT1', [D_IN + 1, N])
    t_xown = din('xT_own', [D_IN + 1, BAND])
    t_win = din('win_rhs', [D_IN + 1, H])
    t_ea = din('eaT1_bf', [ED + 1, EP], BF16)
    t_ew1 = din('ew1_bf', [L, ED + 1, H], BF16)
    t_ew2 = din('ew2_bf', [L, H + 1, H], BF16)
    t_root = din('root_rhs', [L, H + 1, H])
    t_M = din('M_pack_bf', [L, 128, NCH * H], BF16)
    t_lng = din('ln_g', [L, 128, H])
    t_lnb = din('ln_b', [L, 128, H])
    t_deg = din('deg_inv_t', [128, NT])
    t_srci = din('src_idx', [128, ET], I32)
    t_dstl = din('dstloc', [128, ET])
    t_oh = din('oh_own', [128, NT * B])
    t_iota = din('iota', [128, 128])
    t_pw1 = din('pw1_rhs', [H + 1, H])
    t_pw2 = din('pw2_rhs', [H + 1, B])
    t_pw3 = din('pw3_rhs', [B + 1, 1])
    t_out = nc.dram_tensor('pred', [1, B], F32, kind='ExternalOutput')

    ag_space = 'Shared' if shared_ag else 'Local'
    # h tables (bf16): layer 0 written locally by every core; 1..L-1 are
    # AllGather outputs.  Final layer needs no table (pooling is band-local).
    h_bf = [nc.dram_tensor(f'hbf{l}', [N, H], BF16, kind='Internal',
                           addr_space=('Local' if l == 0 else ag_space))
            for l in range(L)]
    band_bf = [nc.dram_tensor(f'bandbf{l}', [BAND, H], BF16, kind='Internal')
               for l in range(L - 1)]
    t_ppart = nc.dram_tensor('ppart', [H, B], F32, kind='Internal')
    t_ppool = nc.dram_tensor('ppool', [H, B], F32, kind='Internal',
                             addr_space=ag_space)

    with tile.TileContext(nc) as tc:
        with tc.tile_pool(name='const', bufs=1) as cp:
            ident = cp.tile([128, 128], F32)
            make_identity(nc, ident[:])
            iota_sb = cp.tile([128, 128], F32)
            nc.sync.dma_start(out=iota_sb[:], in_=t_iota[:, :])
            deg_sb = cp.tile([128, NT], F32)
            nc.sync.dma_start(out=deg_sb[:], in_=t_deg[:, :])
            srci_sb = cp.tile([128, ET], I32)
            nc.sync.dma_start(out=srci_sb[:], in_=t_srci[:, :])
            dstl_sb = cp.tile([128, ET], F32)
            nc.sync.dma_start(out=dstl_sb[:], in_=t_dstl[:, :])
            oh_sb = cp.tile([128, NT * B], F32)
            nc.sync.dma_start(out=oh_sb[:], in_=t_oh[:, :])
            M_sb = [cp.tile([128, NCH * H], BF16, name=f'Msb{l}', tag=f'M{l}')
                    for l in range(L)]
            root_sb = [cp.tile([H + 1, H], F32, name=f'rtsb{l}', tag=f'rt{l}')
                       for l in range(L)]
            lng_sb = [cp.tile([128, H], F32, name=f'lgsb{l}', tag=f'lg{l}')
                      for l in range(L)]
            lnb_sb = [cp.tile([128, H], F32, name=f'lbsb{l}', tag=f'lb{l}')
                      for l in range(L)]
            for l in range(L):
                nc.sync.dma_start(out=M_sb[l][:], in_=t_M[l, :, :])
                nc.sync.dma_start(out=root_sb[l][:], in_=t_root[l, :, :])
                nc.sync.dma_start(out=lng_sb[l][:], in_=t_lng[l, :, :])
                nc.sync.dma_start(out=lnb_sb[l][:], in_=t_lnb[l, :, :])

            # persistent SBUF state
            e2_sb = cp.tile([128, L * ET * H], BF16)   # edge MLP out, all layers
            ind_sb = cp.tile([128, ET * 128], BF16)    # dst indicator tiles
            h_own = [cp.tile([128, NT * H], F32, name=f'hown{i}', tag=f'ho{i}')
                     for i in range(2)]

            # ---- indicators (once; constant across layers) ----
            GRP = 8
            for bq in range(ET // GRP):
                outv = ind_sb[:, bq * GRP * 128:(bq + 1) * GRP * 128].rearrange(
                    'p (t q) -> p t q', t=GRP)
                nc.vector.tensor_tensor(
                    out=outv,
                    in0=dstl_sb[:, bq * GRP:(bq + 1) * GRP][:, :, None]
                        .to_broadcast([128, GRP, 128]),
                    in1=iota_sb[:][:, None, :].to_broadcast([128, GRP, 128]),
                    op=mybir.AluOpType.is_equal)

            # ---- edge MLP -> e2_sb (bf16), all layers ----
            with (tc.tile_pool(name='em', bufs=3) as em,
                  tc.tile_pool(name='emc', bufs=1) as emc,
                  tc.tile_pool(name='emp', bufs=3, space='PSUM') as emp):
                ea_sb = emc.tile([ED + 1, EP], BF16)
                nc.sync.dma_start(out=ea_sb[:], in_=t_ea[:, :])
                e1_sb = emc.tile([H + 1, EP], BF16)
                nc.vector.memset(e1_sb[H:H + 1, :], 1.0)
                for l in range(L):
                    w1 = em.tile([ED + 1, H], BF16, tag='w1')
                    nc.sync.dma_start(out=w1[:], in_=t_ew1[l, :, :])
                    w2 = em.tile([H + 1, H], BF16, tag='w2')
                    nc.sync.dma_start(out=w2[:], in_=t_ew2[l, :, :])
                    for q in range(EP // 512):
                        ps1 = emp.tile([H, 512], F32, tag='p1')
                        nc.tensor.matmul(out=ps1[:],
                                         lhsT=w1[:],
                                         rhs=ea_sb[:, q * 512:(q + 1) * 512],
                                         start=True, stop=True)
                        _leaky(nc, em, e1_sb[0:H, q * 512:(q + 1) * 512], ps1[:],
                               (H, 512))
                    for et in range(ET):
                        ps2 = emp.tile([128, H], F32, tag='p2')
                        nc.tensor.matmul(out=ps2[:],
                                         lhsT=e1_sb[:, et * 128:(et + 1) * 128],
                                         rhs=w2[:], start=True, stop=True)
                        _leaky(nc, em,
                               e2_sb[:, (l * ET + et) * H:(l * ET + et + 1) * H],
                               ps2[:], (128, H))

            # ---- input projection ----
            with (tc.tile_pool(name='s0', bufs=4) as s0,
                  tc.tile_pool(name='s0c', bufs=1) as s0c,
                  tc.tile_pool(name='s0p', bufs=4, space='PSUM') as s0p):
                xT_sb = s0c.tile([D_IN + 1, N], F32)
                nc.sync.dma_start(out=xT_sb[:], in_=t_xT1[:, :])
                xo_sb = s0c.tile([D_IN + 1, BAND], F32)
                nc.sync.dma_start(out=xo_sb[:], in_=t_xown[:, :])
                win_sb = s0c.tile([D_IN + 1, H], F32)
                nc.sync.dma_start(out=win_sb[:], in_=t_win[:, :])
                # full bf16 table (all cores compute all groups locally)
                for g8 in range(N // 128 // 8):
                    hstage = s0.tile([128, 8 * H], BF16, tag='hstage')
                    for k in range(8):
                        g = g8 * 8 + k
                        ps = s0p.tile([128, H], F32, tag='p')
                        nc.tensor.matmul(out=ps[:],
                                         lhsT=xT_sb[:, g * 128:(g + 1) * 128],
                                         rhs=win_sb[:], start=True, stop=True)
                        _leaky(nc, s0, hstage[:, k * H:(k + 1) * H], ps[:],
                               (128, H))
                    # half-band-major remap: nodes g8*1024.. live at block
                    # (g8%2)*N/2 + (g8//2)*1024 of the h table
                    base = (g8 % 2) * (N // 2) + (g8 // 2) * 1024
                    nc.sync.dma_start(
                        out=h_bf[0][base:base + 1024, :].rearrange(
                            '(j p) d -> p j d', p=128),
                        in_=hstage[:].rearrange('p (j d) -> p j d', j=8))
                # own band fp32 (residual/root path)
                for nt in range(NT):
                    ps = s0p.tile([128, H], F32, tag='p')
                    nc.tensor.matmul(out=ps[:],
                                     lhsT=xo_sb[:, nt * 128:(nt + 1) * 128],
                                     rhs=win_sb[:], start=True, stop=True)
                    _leaky(nc, s0, h_own[0][:, nt * H:(nt + 1) * H], ps[:],
                           (128, H))

            # ---- layers ----
            with (tc.tile_pool(name='zz', bufs=2) as zp,
                  tc.tile_pool(name='hsp', bufs=3) as hsp,
                  tc.tile_pool(name='stp', bufs=3) as stp,
                  tc.tile_pool(name='lyn', bufs=3) as lyn,
                  tc.tile_pool(name='stg', bufs=2) as stg,
                  tc.tile_pool(name='pS', bufs=3, space='PSUM') as pS,
                  tc.tile_pool(name='pHs', bufs=1, space='PSUM') as pHs,
                  tc.tile_pool(name='pag', bufs=2, space='PSUM') as pag,
                  tc.tile_pool(name='ptp', bufs=1, space='PSUM') as ptp,
                  tc.tile_pool(name='pag2', bufs=1, space='PSUM') as pag2):
                for l in range(L):
                    hin = h_own[l % 2]
                    hout = h_own[(l + 1) % 2]
                    hs_t = [None] * NT

                    def gather(nt):
                        hs = hsp.tile([128, T_et * H], BF16, tag='hs')
                        nc.gpsimd.indirect_dma_start(
                            out=hs[:], out_offset=None,
                            in_=h_bf[l][:, :],
                            in_offset=bass.IndirectOffsetOnAxis(
                                ap=srci_sb[:, nt * T_et:(nt + 1) * T_et],
                                axis=0))
                        hs_t[nt] = hs

                    gather(0)
                    for nt in range(NT):
                        if nt + 1 < NT:
                            gather(nt + 1)
                        hs = hs_t[nt]
                        z = zp.tile([128, T_et * H * H], BF16, tag='z')
                        for j in range(T_et):
                            zv = z[:, j * H * H:(j + 1) * H * H].rearrange(
                                'p (i k) -> p i k', i=H)
                            e2ap = e2_sb[:, (l * ET + nt * T_et + j) * H:
                                         (l * ET + nt * T_et + j + 1) * H]
                            nc.vector.tensor_tensor(
                                out=zv[:, 0:SP_ROWS, :],
                                in0=e2ap[:, None, :]
                                    .to_broadcast([128, SP_ROWS, H]),
                                in1=hs[:, j * H:j * H + SP_ROWS]
                                    .to_broadcast([128, SP_ROWS, H]),
                                op=mybir.AluOpType.mult)
                            nc.gpsimd.tensor_tensor(
                                out=zv[:, SP_ROWS:H, :],
                                in0=e2ap[:, None, :]
                                    .to_broadcast([128, H - SP_ROWS, H]),
                                in1=hs[:, j * H + SP_ROWS:(j + 1) * H]
                                    .to_broadcast([128, H - SP_ROWS, H]),
                                op=mybir.AluOpType.mult)

                        # scatter (transposed): 8 pass-tiles x 4 chunks, then Hs
                        agg = pag.tile([128, H], F32, tag='agg')
                        sp_t, st_t = [None] * 8, [None] * 8

                        def scatter_pass(pt):
                            sp = pS.tile([128, 512], F32, tag='sp')
                            for c4 in range(4):
                                q = pt * 4 + c4
                                for j in range(T_et):
                                    nc.tensor.matmul(
                                        out=sp[:, c4 * 128:(c4 + 1) * 128],
                                        lhsT=z[:, (j * H * H + q * 128):
                                               (j * H * H + q * 128 + 128)],
                                        rhs=ind_sb[:, (nt * T_et + j) * 128:
                                                   (nt * T_et + j + 1) * 128],
                                        start=(j == 0), stop=(j == T_et - 1))
                            sp_t[pt] = sp

                        def drain_pass(pt):
                            st = stp.tile([128, 512], BF16, tag='st')
                            nc.scalar.copy(out=st[:], in_=sp_t[pt][:])
                            for c4 in range(4):
                                q = pt * 4 + c4
                                nc.tensor.matmul(
                                    out=agg[:],
                                    lhsT=st[:, c4 * 128:(c4 + 1) * 128],
                                    rhs=M_sb[l][:, q * H:(q + 1) * H],
                                    start=(q == 0), stop=False)

                        scatter_pass(0)
                        for pt in range(1, 8):
                            scatter_pass(pt)
                            drain_pass(pt - 1)
                        drain_pass(7)
                        # Hs chunk (q=32): HsT[i,n] = sum_e hs[e,i] ind[e,n]
                        hq = pHs.tile([H, 128], F32, tag='hq')
                        for j in range(T_et):
                            nc.tensor.matmul(
                                out=hq[:],
                                lhsT=hs[:, j * H:(j + 1) * H],
                                rhs=ind_sb[:, (nt * T_et + j) * 128:
                                           (nt * T_et + j + 1) * 128],
                                start=(j == 0), stop=(j == T_et - 1))
                        sth = stp.tile([H, 128], BF16, tag='sth')
                        nc.scalar.copy(out=sth[:], in_=hq[:])
                        nc.tensor.matmul(
                            out=agg[:], lhsT=sth[:],
                            rhs=M_sb[l][0:H, (NCH - 1) * H:NCH * H],
                            start=False, stop=True)

                        # root: transpose own h tile, matmul with root weights
                        htp = ptp.tile([H, 128], F32, tag='tp')
                        nc.tensor.transpose(out=htp[:],
                                            in_=hin[:, nt * H:(nt + 1) * H],
                                            identity=ident[:])
                        hoT = lyn.tile([H + 1, 128], F32, tag='hoT')
                        nc.scalar.copy(out=hoT[0:H, :], in_=htp[:])
                        nc.vector.memset(hoT[H:H + 1, :], 1.0)
                        ag2 = pag2.tile([128, H], F32, tag='ag2')
                        nc.tensor.matmul(out=ag2[:], lhsT=hoT[:],
                                         rhs=root_sb[l][:],
                                         start=True, stop=True)

                        # node update: out = agg*deg_inv + root; LN; leaky; +h
                        xs = lyn.tile([128, H], F32, tag='xs')
                        nc.vector.tensor_scalar(
                            out=xs[:], in0=agg[:], scalar1=deg_sb[:, nt:nt + 1],
                            scalar2=None, op0=mybir.AluOpType.mult)
                        nc.vector.tensor_tensor(out=xs[:], in0=xs[:], in1=ag2[:],
                                                op=mybir.AluOpType.add)
                        mu = lyn.tile([128, 1], F32, tag='mu')
                        nc.vector.tensor_reduce(out=mu[:], in_=xs[:],
                                                axis=mybir.AxisListType.X,
                                                op=mybir.AluOpType.add)
                        nc.vector.tensor_scalar_mul(mu[:], mu[:], 1.0 / H)
                        xc = lyn.tile([128, H], F32, tag='xc')
                        nc.vector.tensor_scalar(
                            out=xc[:], in0=xs[:], scalar1=mu[:], scalar2=None,
                            op0=mybir.AluOpType.subtract)
                        sq = lyn.tile([128, H], F32, tag='sq')
                        nc.vector.tensor_tensor(out=sq[:], in0=xc[:], in1=xc[:],
                                                op=mybir.AluOpType.mult)
                        vs = lyn.tile([128, 1], F32, tag='vs')
                        nc.vector.tensor_reduce(out=vs[:], in_=sq[:],
                                                axis=mybir.AxisListType.X,
                                                op=mybir.AluOpType.add)
                        nc.vector.tensor_scalar(
                            out=vs[:], in0=vs[:], scalar1=1.0 / H, scalar2=EPS,
                            op0=mybir.AluOpType.mult, op1=mybir.AluOpType.add)
                        sd = lyn.tile([128, 1], F32, tag='sd')
                        nc.scalar.activation(out=sd[:], in_=vs[:],
                                             func=mybir.ActivationFunctionType.Sqrt)
                        rs = lyn.tile([128, 1], F32, tag='rs')
                        nc.vector.reciprocal(out=rs[:], in_=sd[:])
                        yv = lyn.tile([128, H], F32, tag='yv')
                        nc.vector.tensor_scalar(
                            out=yv[:], in0=xc[:], scalar1=rs[:], scalar2=None,
                            op0=mybir.AluOpType.mult)
                        nc.vector.tensor_tensor(out=yv[:], in0=yv[:],
                                                in1=lng_sb[l][:],
                                                op=mybir.AluOpType.mult)
                        nc.vector.tensor_tensor(out=yv[:], in0=yv[:],
                                                in1=lnb_sb[l][:],
                                                op=mybir.AluOpType.add)
                        hn = lyn.tile([128, H], F32, tag='hn')
                        _leaky(nc, lyn, hn[:], yv[:], (128, H))
                        nc.vector.tensor_tensor(
                            out=hout[:, nt * H:(nt + 1) * H], in0=hn[:],
                            in1=hin[:, nt * H:(nt + 1) * H],
                            op=mybir.AluOpType.add)
                        if l < L - 1:
                            # bf16 staging for the AllGather band
                            if nt % 8 == 0:
                                bstage = stg.tile([128, 8 * H], BF16,
                                                  tag='bstage')
                            nc.scalar.copy(
                                out=bstage[:, (nt % 8) * H:(nt % 8 + 1) * H],
                                in_=hout[:, nt * H:(nt + 1) * H])
                            if nt % 8 == 7:
                                half = nt // 8
                                nc.sync.dma_start(
                                    out=band_bf[l][half * 1024:
                                                   (half + 1) * 1024, :]
                                    .rearrange('(j p) d -> p j d', p=128),
                                    in_=bstage[:].rearrange(
                                        'p (j d) -> p j d', j=8))
                                # AllGather this half into the (contiguous)
                                # half-band-major block of the next h table;
                                # issued on the SP queue right after the band
                                # write so it overlaps the other half's
                                # compute without blocking Pool gathers.
                                nc.gpsimd.collective_compute(
                                    'AllGather', mybir.AluOpType.bypass,
                                    replica_groups=[list(range(NC))],
                                    ins=[band_bf[l][half * 1024:
                                                    (half + 1) * 1024,
                                                    :].opt()],
                                    outs=[h_bf[l + 1][half * (N // 2):
                                                      (half + 1) * (N // 2),
                                                      :].opt()])

            # ---- pool (own band partial + AllReduce) + head ----
            with (tc.tile_pool(name='hd', bufs=2) as hd,
                  tc.tile_pool(name='hdc', bufs=1) as hdc,
                  tc.tile_pool(name='hdp', bufs=2, space='PSUM') as hdp):
                hfin = h_own[L % 2]
                pool_ps = hdp.tile([H, B], F32, tag='pool')
                for nt in range(NT):
                    nc.tensor.matmul(out=pool_ps[:],
                                     lhsT=hfin[:, nt * H:(nt + 1) * H],
                                     rhs=oh_sb[:, nt * B:(nt + 1) * B],
                                     start=(nt == 0), stop=(nt == NT - 1))
                ppsb = hdc.tile([H, B], F32)
                nc.scalar.copy(out=ppsb[:], in_=pool_ps[:])
                nc.sync.dma_start(out=t_ppart[:, :], in_=ppsb[:])
                nc.gpsimd.collective_compute(
                    'AllReduce', mybir.AluOpType.add,
                    replica_groups=[list(range(NC))],
                    ins=[t_ppart[:, :].opt()],
                    outs=[t_ppool[:, :].opt()])
                pT = hdc.tile([H + 1, B], F32)
                nc.sync.dma_start(out=pT[0:H, :], in_=t_ppool[:, :])
                nc.vector.memset(pT[H:H + 1, :], 1.0)
                w1 = hdc.tile([H + 1, H], F32)
                nc.sync.dma_start(out=w1[:], in_=t_pw1[:, :])
                w2 = hdc.tile([H + 1, B], F32)
                nc.sync.dma_start(out=w2[:], in_=t_pw2[:, :])
                w3 = hdc.tile([B + 1, 1], F32)
                nc.sync.dma_start(out=w3[:], in_=t_pw3[:, :])
                p1ps = hdp.tile([H, B], F32, tag='p1')
                nc.tensor.matmul(out=p1ps[:], lhsT=w1[:], rhs=pT[:],
                                 start=True, stop=True)
                p1 = hdc.tile([H + 1, B], F32)
                _leaky(nc, hd, p1[0:H, :], p1ps[:], (H, B))
                nc.vector.memset(p1[H:H + 1, :], 1.0)
                p2ps = hdp.tile([B, B], F32, tag='p2')
                nc.tensor.matmul(out=p2ps[:], lhsT=w2[:], rhs=p1[:],
                                 start=True, stop=True)
                p2 = hdc.tile([B + 1, B], F32)
                _leaky(nc, hd, p2[0:B, :], p2ps[:], (B, B))
                nc.vector.memset(p2[B:B + 1, :], 1.0)
                p3ps = hdp.tile([1, B], F32, tag='p3')
                nc.tensor.matmul(out=p3ps[:], lhsT=w3[:], rhs=p2[:],
                                 start=True, stop=True)
                pr = hdc.tile([1, B], F32)
                nc.scalar.copy(out=pr[:], in_=p3ps[:])
                nc.sync.dma_start(out=t_out[:, :], in_=pr[:])

    nc.compile()
    return nc


_CACHE = {}


def kernel(**inputs) -> np.ndarray:
    host = _host_prep(inputs)
    T_et, EP = host['T_et'], host['EP']
    key = (T_et, EP)
    if key not in _CACHE:
        _CACHE[key] = _build(T_et, EP)
    nc = _CACHE[key]
    in_maps = make_in_maps(host)
    res = bass_utils.run_bass_kernel_spmd(nc, in_maps, core_ids=list(range(NC)))
    return np.asarray(res.results[0]['pred'][0], np.float32)
